# revision 1
# baseline (speedup 1.0000x reference)
"""Trainium2 Bass kernel for GQA attention with QK-RMSNorm, partial mRoPE,
causal mask and sigmoid output gate (nn_Attention_70557722739259).

Model dims: B=2, T=2048, D=2048, N=16 Q heads, K=2 KV heads, H=256.
Sharding over 8 NeuronCores: DP=2 over batch x TP=4 over head groups
(4 Q heads + their shared KV head per core). Each core computes a partial
output projection over its 4 heads; the host sums the 4 partials per batch
(Megatron-style unshard).

Per-core device algorithm (all matmuls bf16, softmax f32):
  phase 1 (two half-T passes): projections q/gate/k/v from host-pre-transposed
           x^T, RMSNorm stats via DVE tensor_tensor_reduce, sigmoid gate,
           batched Sqrt for rms, RoPE + norm scale token-major, PE-transpose
           Q,K to feature-major (H, T).
  phase 2: per head, S^T = K^T.T @ Q^T tiles (keys on partitions), exp via
           ACT (1/16 scale folded in; no max subtraction - scores are O(10)),
           multiplicative 0/1 causal mask on diagonal blocks, AV + softmax
           denominator accumulated in PSUM via ones-column in V.
  phase 3: o-proj partial from gated qkv^T (feature-major via PE transpose).
"""
import sys
sys.path.insert(0, "/opt/trn_rl_repo")
import numpy as np
import ml_dtypes

from concourse import bacc, tile, mybir
from concourse import bass_utils
from concourse.masks import make_identity

BF16 = ml_dtypes.bfloat16
F32 = mybir.dt.float32
BF = mybir.dt.bfloat16

B, T, D = 2, 2048, 2048
N_HEADS, N_KV, H = 16, 2, 256
HEADS_PC = 4            # q heads per core (TP=4)
ROPE_THETA = 1000000
ROTARY = 64             # int(H * 0.25)
FREQ = 32
NORM_EPS = 1e-6
K_MASK = -2.3819763e38
SCALE = H ** (-0.5)     # 1/16

TT = T // 128           # 16 token tiles
DC = D // 128           # 16 contraction chunks

LAST_RESULT = None
LAST_IN_MAPS = None
_COMPILED = {}


def _build(mode="causal", apply_w=False, phases=(1, 2, 3), n_halves=2, n_heads_dbg=HEADS_PC):
    nc = bacc.Bacc("TRN2", target_bir_lowering=False, debug=False,
                   enable_asserts=True, num_devices=8)
    Act = mybir.ActivationFunctionType
    Alu = mybir.AluOpType

    xT = nc.dram_tensor("xT", (D, T), BF, kind="ExternalInput").ap()
    wq = nc.dram_tensor("wq", (D, HEADS_PC * 2 * H), BF, kind="ExternalInput").ap()
    wkv = nc.dram_tensor("wkv", (D, 2 * H), BF, kind="ExternalInput").ap()
    wo = nc.dram_tensor("wo", (HEADS_PC * H, D), BF, kind="ExternalInput").ap()
    cc = nc.dram_tensor("cc", (T, ROTARY), BF, kind="ExternalInput").ap()
    ss = nc.dram_tensor("ss", (T, ROTARY), BF, kind="ExternalInput").ap()
    if apply_w:
        qw = nc.dram_tensor("qw", (128, H), F32, kind="ExternalInput").ap()
        kw = nc.dram_tensor("kw", (128, H), F32, kind="ExternalInput").ap()
    if mode == "arbitrary":
        am = nc.dram_tensor("am", (T, T), F32, kind="ExternalInput").ap()
    out = nc.dram_tensor("out", (T, D), BF, kind="ExternalOutput").ap()

    with tile.TileContext(nc) as tc:
        with tc.tile_pool(name="const", bufs=1) as constp, \
             tc.tile_pool(name="attn", bufs=1) as ap_, \
             tc.tile_pool(name="psum", bufs=1, space="PSUM") as psum:

            # ---- constants ----
            ident = constp.tile([128, 128], BF, tag="ident", name="ident")
            make_identity(nc, ident[:])
            epst = constp.tile([128, 1], F32, tag="epst", name="epst")
            nc.gpsimd.memset(epst[:], NORM_EPS)
            tri = []
            if mode == "causal":
                for r in range(4):
                    t = constp.tile([128, 512], BF, tag=f"tri{r}", name=f"tri{r}")
                    nc.gpsimd.memset(t[:], 1.0)
                    # keep (1.0) where col - part - 128*r >= 0, else fill 0.0
                    nc.gpsimd.affine_select(
                        out=t[:], in_=t[:], compare_op=Alu.is_ge, fill=0.0,
                        base=-128 * r, channel_multiplier=-1, pattern=[[1, 512]])
                    tri.append(t)
            if apply_w:
                qw_sb = constp.tile([128, H], F32, tag="qw", name="qw")
                kw_sb = constp.tile([128, H], F32, tag="kw", name="kw")
                nc.sync.dma_start(qw_sb[:], qw[:])
                nc.sync.dma_start(kw_sb[:], kw[:])

            # ---- persistent attention tensors (span phase 1 -> 2) ----
            QT = [[ap_.tile([128, T], BF, tag=f"QT{h}_{c}", name=f"QT{h}_{c}")
                   for c in range(2)] for h in range(HEADS_PC)]
            KT = [ap_.tile([128, T], BF, tag=f"KT{c}", name=f"KT{c}")
                  for c in range(2)]
            V = [ap_.tile([128, H + 1], BF, tag=f"V{i}", name=f"V{i}")
                 for i in range(TT)]
            gate = [[ap_.tile([128, H], BF, tag=f"g{h}_{i}", name=f"g{h}_{i}")
                     for i in range(TT)] for h in range(HEADS_PC)]

            # ================= phase 1 (two half-T passes) =================
            with tc.tile_pool(name="praw", bufs=1) as praw, \
                 tc.tile_pool(name="proj", bufs=1) as proj, \
                 tc.tile_pool(name="p1w", bufs=3) as p1w, \
                 tc.tile_pool(name="p1c", bufs=3) as p1c:
                wkv_sb = [proj.tile([128, 2 * H], BF, tag=f"wkv{d}",
                                    name=f"wkv{d}") for d in range(DC)]
                for d in range(DC):
                    nc.sync.dma_start(wkv_sb[d][:],
                                      wkv[d * 128:(d + 1) * 128, :])

                n_half = HEADS_PC * 8 + 8   # ssq columns per half (q then k)

                for half in range(n_halves):
                    t0 = half * 1024
                    xT_sb = []
                    for d in range(DC):
                        xt_ = proj.tile([128, 1024], BF, tag=f"xT{d}",
                                        name=f"xT{d}_{half}")
                        nc.sync.dma_start(
                            xt_[:], xT[d * 128:(d + 1) * 128, t0:t0 + 1024])
                        xT_sb.append(xt_)
                    cc_sb, ss_sb = [], []
                    for tl in range(8):
                        ti = half * 8 + tl
                        c_ = praw.tile([128, ROTARY], BF, tag=f"cc{tl}",
                                       name=f"cc{tl}_{half}")
                        s_ = praw.tile([128, ROTARY], BF, tag=f"ss{tl}",
                                       name=f"ss{tl}_{half}")
                        nc.sync.dma_start(c_[:], cc[ti * 128:(ti + 1) * 128, :])
                        nc.sync.dma_start(s_[:], ss[ti * 128:(ti + 1) * 128, :])
                        cc_sb.append(c_)
                        ss_sb.append(s_)

                    ssqall = praw.tile([128, n_half], F32, tag="ssq", bufs=2,
                                       name=f"ssq{half}")
                    q_raw = [[praw.tile([128, H], BF, tag=f"qr{h}_{tl}",
                                        name=f"qr{h}_{tl}_{half}")
                              for tl in range(8)] for h in range(HEADS_PC)]
                    k_raw = [praw.tile([128, H], BF, tag=f"kr{tl}",
                                       name=f"kr{tl}_{half}")
                             for tl in range(8)]

                    # ---- pass A: projections ----
                    for tl in range(8):
                        ti = half * 8 + tl
                        pk = psum.tile([128, 2 * H], F32, tag="mm", bufs=2,
                                       name=f"pk{ti}")
                        for d in range(DC):
                            nc.tensor.matmul(
                                pk[:], xT_sb[d][:, tl * 128:(tl + 1) * 128],
                                wkv_sb[d][:], start=(d == 0), stop=(d == DC - 1))
                        kvt = p1w.tile([128, 2 * H], BF, tag="fulltmp",
                                       name=f"kvt{ti}")
                        nc.scalar.copy(kvt[:], pk[:])
                        nc.vector.tensor_copy(k_raw[tl][:], kvt[:, 0:H])
                        nc.vector.tensor_copy(V[ti][:, 0:H], kvt[:, H:2 * H])
                        nc.gpsimd.memset(V[ti][:, H:H + 1], 1.0)
                        junk = p1w.tile([128, H], F32, tag="junk", name=f"jk{ti}")
                        nc.scalar.square(junk[:], k_raw[tl][:])
                        nc.vector.reduce_sum(ssqall[:, 32 + tl:32 + tl + 1],
                                             junk[:], axis=mybir.AxisListType.X)

                    for h in range(n_heads_dbg):
                        wq_h = []
                        for d in range(DC):
                            w_ = proj.tile([128, 512], BF, tag=f"wq{d}",
                                           name=f"wq{d}_{half}_{h}")
                            nc.sync.dma_start(
                                w_[:], wq[d * 128:(d + 1) * 128,
                                          h * 512:(h + 1) * 512])
                            wq_h.append(w_)
                        for tl in range(8):
                            ti = half * 8 + tl
                            pq = psum.tile([128, 2 * H], F32, tag="mm", bufs=2,
                                           name=f"pq{h}_{ti}")
                            for d in range(DC):
                                nc.tensor.matmul(
                                    pq[:], xT_sb[d][:, tl * 128:(tl + 1) * 128],
                                    wq_h[d][:], start=(d == 0),
                                    stop=(d == DC - 1))
                            qgt = p1w.tile([128, 2 * H], BF, tag="fulltmp",
                                           name=f"qgt{h}_{ti}")
                            nc.scalar.copy(qgt[:], pq[:])
                            nc.vector.tensor_copy(q_raw[h][tl][:], qgt[:, 0:H])
                            nc.scalar.activation(gate[h][ti][:], qgt[:, H:2 * H],
                                                 Act.Sigmoid)
                            junk = p1w.tile([128, H], F32, tag="junk",
                                            name=f"jq{h}_{ti}")
                            nc.scalar.square(junk[:], q_raw[h][tl][:])
                            nc.vector.reduce_sum(
                                ssqall[:, h * 8 + tl:h * 8 + tl + 1],
                                junk[:], axis=mybir.AxisListType.X)

                    # ---- pass B: batched rms scales for this half ----
                    rms = praw.tile([128, n_half], F32, tag="rms", bufs=2,
                                    name=f"rms{half}")
                    rinv = praw.tile([128, n_half], F32, tag="rinv", bufs=2,
                                     name=f"rinv{half}")
                    nc.scalar.activation(rms[:], ssqall[:], Act.Sqrt,
                                         scale=1.0 / H, bias=epst[:])
                    nc.vector.reciprocal(rinv[:], rms[:])

                    # ---- pass C: rope + norm scale + transpose ----
                    def rope_pass(raw, col, dst_tiles, tl, w_sb, nm):
                        ti = half * 8 + tl
                        src = raw
                        if apply_w:
                            srw = p1c.tile([128, H], F32, tag="srw",
                                           name=f"srw{nm}")
                            nc.vector.tensor_mul(srw[:], raw[:], w_sb[:])
                            src = srw
                        rot = p1c.tile([128, ROTARY], BF, tag="rot",
                                       name=f"rot{nm}")
                        t2 = p1c.tile([128, ROTARY], BF, tag="rot2",
                                      name=f"rot2{nm}")
                        nc.vector.tensor_mul(rot[:], src[:, 0:ROTARY],
                                             cc_sb[tl][:])
                        nc.vector.tensor_mul(t2[:, 0:FREQ], src[:, FREQ:ROTARY],
                                             ss_sb[tl][:, 0:FREQ])
                        nc.vector.tensor_mul(t2[:, FREQ:ROTARY], src[:, 0:FREQ],
                                             ss_sb[tl][:, FREQ:ROTARY])
                        nc.vector.tensor_add(rot[:], rot[:], t2[:])
                        tok = p1c.tile([128, H], BF, tag="tok", name=f"tok{nm}")
                        rv = rinv[:, col:col + 1]
                        nc.vector.tensor_scalar_mul(tok[:, 0:ROTARY], rot[:], rv)
                        nc.vector.tensor_scalar_mul(tok[:, ROTARY:H],
                                                    src[:, ROTARY:H], rv)
                        for c2 in range(2):
                            tp = psum.tile([128, 128], BF, tag="tp", bufs=2,
                                           name=f"tp{nm}_{c2}")
                            nc.tensor.transpose(
                                tp[:], tok[:, c2 * 128:(c2 + 1) * 128], ident[:])
                            nc.any.tensor_copy(
                                dst_tiles[c2][:, ti * 128:(ti + 1) * 128], tp[:])

                    for tl in range(8):
                        rope_pass(k_raw[tl], 32 + tl, KT, tl,
                                  kw_sb if apply_w else None, f"k{half}_{tl}")
                    for h in range(n_heads_dbg):
                        for tl in range(8):
                            rope_pass(q_raw[h][tl], h * 8 + tl, QT[h], tl,
                                      qw_sb if apply_w else None,
                                      f"q{h}_{half}_{tl}")

            # ================= phases 2+3 =================
            if phases == (1,):
                # debug: dump K^T and QT[0] so HW output is defined
                with tc.tile_pool(name="dbg", bufs=2) as dbg:
                    for idx, tile_ in enumerate([KT[0], KT[1], QT[0][0],
                                                 QT[0][1], QT[1][0], QT[1][1]]):
                        dt_ = dbg.tile([128, T], BF, tag="dbg", name=f"dbg{idx}")
                        nc.vector.tensor_copy(dt_[:], tile_[:])
                        nc.sync.dma_start(out[idx * 128:(idx + 1) * 128, :], dt_[:])
                    for idx2 in range(6, 16):
                        ti2 = idx2 - 6
                        dt_ = dbg.tile([128, T], BF, tag="dbg", name=f"dbg{idx2}")
                        nc.gpsimd.memset(dt_[:], 0.0)
                        nc.vector.tensor_copy(dt_[:, 0:H], V[ti2][:, 0:H])
                        nc.vector.tensor_copy(dt_[:, H:2 * H], gate[0][ti2][:])
                        nc.sync.dma_start(out[idx2 * 128:(idx2 + 1) * 128, :],
                                          dt_[:])
            if phases != (1,):
              with tc.tile_pool(name="p23", bufs=1) as p23, \
                 tc.tile_pool(name="p2w", bufs=3) as p2w:
                  qkvgT = [p23.tile([128, T], BF, tag=f"qkT{c}", name=f"qkT{c}")
                           for c in range(2 * HEADS_PC)]
                  wo_sb = [p23.tile([128, D], BF, tag=f"wo{c}", name=f"wo{c}")
                           for c in range(2 * HEADS_PC)]
                  for c in range(2 * HEADS_PC):
                      nc.sync.dma_start(wo_sb[c][:], wo[c * 128:(c + 1) * 128, :])

                  for j in range(4):              # q blocks of 512
                    kmax = 4 * (j + 1) if mode == "causal" else TT
                    for h in range(HEADS_PC):
                          av = [psum.tile([128, H + 1], F32, tag=f"av{s}", bufs=1,
                                          name=f"av{h}_{j}_{s}") for s in range(4)]
                          for i in range(kmax):   # key chunks of 128
                              st = psum.tile([128, 512], F32, tag="mm", bufs=2,
                                             name=f"st{h}_{j}_{i}")
                              for c2 in range(2):
                                  nc.tensor.matmul(
                                      st[:],
                                      KT[c2][:, i * 128:(i + 1) * 128],
                                      QT[h][c2][:, j * 512:(j + 1) * 512],
                                      start=(c2 == 0), stop=(c2 == 1))
                              if mode == "arbitrary":
                                  amt = p2w.tile([128, 512], F32, tag="amt",
                                                 name=f"am{h}_{j}_{i}")
                                  nc.sync.dma_start(
                                      amt[:], am[i * 128:(i + 1) * 128,
                                                 j * 512:(j + 1) * 512])
                                  nc.vector.tensor_add(st[:], st[:], amt[:])
                              pT = p2w.tile([128, 512], BF, tag="pT",
                                            name=f"pT{h}_{j}_{i}")
                              nc.scalar.activation(pT[:], st[:], Act.Exp,
                                                   scale=SCALE)
                              if mode == "causal" and i >= 4 * j:
                                  nc.vector.tensor_mul(pT[:], pT[:],
                                                       tri[i - 4 * j][:])
                              for s in range(4):
                                  last_i = (4 * j + s) if mode == "causal" \
                                      else (kmax - 1)
                                  if i > last_i:
                                      continue
                                  nc.tensor.matmul(
                                      av[s][:], pT[:, s * 128:(s + 1) * 128],
                                      V[i][:], start=(i == 0),
                                      stop=(i == last_i))
                          for s in range(4):
                              ti = 4 * j + s
                              avs = p2w.tile([128, H + 1], F32, tag="avs",
                                             name=f"avs{h}_{ti}")
                              nc.vector.tensor_copy(avs[:], av[s][:])
                              rec = p2w.tile([128, 1], F32, tag="rec",
                                             name=f"rec{h}_{ti}")
                              if mode == "arbitrary":
                                  dcl = p2w.tile([128, 1], F32, tag="dcl",
                                                 name=f"dcl{h}_{ti}")
                                  nc.vector.tensor_scalar_max(
                                      dcl[:], avs[:, H:H + 1], 1e-30)
                                  nc.vector.reciprocal(rec[:], dcl[:])
                              else:
                                  nc.vector.reciprocal(rec[:], avs[:, H:H + 1])
                              tmp = p2w.tile([128, H], BF, tag="avt",
                                             name=f"avt{h}_{ti}")
                              nc.vector.tensor_scalar_mul(tmp[:], avs[:, 0:H],
                                                          rec[:])
                              qk = p2w.tile([128, H], BF, tag="qkg",
                                            name=f"qkg{h}_{ti}")
                              nc.vector.tensor_mul(qk[:], tmp[:], gate[h][ti][:])
                              for c2 in range(2):
                                  tp = psum.tile([128, 128], BF, tag="tp", bufs=2,
                                                 name=f"tp2{h}_{ti}_{c2}")
                                  nc.tensor.transpose(
                                      tp[:], qk[:, c2 * 128:(c2 + 1) * 128],
                                      ident[:])
                                  nc.any.tensor_copy(
                                      qkvgT[2 * h + c2][:, ti * 128:(ti + 1) * 128],
                                      tp[:])
                    # ---- phase 3 for this token block (overlaps next j) ----
                    if 3 in phases:
                      for ti in range(4 * j, 4 * (j + 1)):
                          for db in range(4):
                              po = psum.tile([128, 512], F32, tag="mm", bufs=2,
                                             name=f"po{ti}_{db}")
                              for c in range(2 * HEADS_PC):
                                  nc.tensor.matmul(
                                      po[:], qkvgT[c][:, ti * 128:(ti + 1) * 128],
                                      wo_sb[c][:, db * 512:(db + 1) * 512],
                                      start=(c == 0), stop=(c == 2 * HEADS_PC - 1))
                              ot = p2w.tile([128, 512], BF, tag="ot",
                                            name=f"ot{ti}_{db}")
                              nc.any.tensor_copy(ot[:], po[:])
                              nc.sync.dma_start(
                                  out[ti * 128:(ti + 1) * 128,
                                      db * 512:(db + 1) * 512], ot[:])
                  if 3 not in phases:
                      for c in range(2 * HEADS_PC):
                          dt_ = p2w.tile([128, T], BF, tag="dbg3", name=f"dbg3{c}")
                          nc.vector.tensor_copy(dt_[:], qkvgT[c][:])
                          nc.sync.dma_start(out[c * 128:(c + 1) * 128, :], dt_[:])

    nc.compile()
    return nc


def _get_compiled(mode, apply_w):
    key = (mode, apply_w)
    if key not in _COMPILED:
        _COMPILED[key] = _build(mode, apply_w)
    return _COMPILED[key]


def _rope_tables(positions):
    """Host: exact reference mRoPE sin/cos tables -> CC=[cos|cos], SS=[-sin|sin]."""
    fraction = 2.0 * np.arange(FREQ, dtype=np.float32) / ROTARY
    timescale = (ROPE_THETA ** fraction).astype(np.float32)
    CC, SS = [], []
    for b in range(positions.shape[1]):
        sinusoid = positions[:, b, :, None].astype(np.float32) / timescale
        freq = sinusoid[0].copy()
        h_idx = np.arange(1, 11 * 3, 3)
        w_idx = np.arange(2, 10 * 3, 3)
        freq[:, h_idx] = sinusoid[1][:, h_idx]
        freq[:, w_idx] = sinusoid[2][:, w_idx]
        sin, cos = np.sin(freq), np.cos(freq)
        CC.append(np.concatenate([cos, cos], axis=1).astype(np.float32))
        SS.append(np.concatenate([-sin, sin], axis=1).astype(np.float32))
    return CC, SS


def kernel(x, positions, attn_mask, wq, wk, wv, wo, q_norm_w, k_norm_w):
    global LAST_RESULT, LAST_IN_MAPS
    x = np.asarray(x)
    positions = np.asarray(positions)
    attn_mask = np.asarray(attn_mask)
    wq, wk, wv, wo = map(np.asarray, (wq, wk, wv, wo))
    q_norm_w, k_norm_w = np.asarray(q_norm_w), np.asarray(k_norm_w)

    tril = np.tril(np.ones((T, T), dtype=bool))
    if all(np.array_equal(attn_mask[b], tril) for b in range(B)):
        mode = "causal"
    elif attn_mask.all():
        mode = "full"
    else:
        mode = "arbitrary"
    apply_w = bool(np.any(q_norm_w != 0) or np.any(k_norm_w != 0))

    nc = _get_compiled(mode, apply_w)
    CC, SS = _rope_tables(positions)
    group = N_HEADS // N_KV  # q heads per kv head = 8

    in_maps = []
    for c in range(8):
        b, g = c // 4, c % 4
        kvh = (g * HEADS_PC) // group
        m = {
            "xT": np.ascontiguousarray(x[b].T).astype(BF16),
            "wq": np.ascontiguousarray(
                wq[:, g * HEADS_PC:(g + 1) * HEADS_PC, :]).reshape(
                    D, HEADS_PC * 2 * H).astype(BF16),
            "wkv": np.ascontiguousarray(np.concatenate(
                [wk[:, kvh, :], wv[:, kvh, :]], axis=1)).astype(BF16),
            "wo": np.ascontiguousarray(
                wo[g * HEADS_PC:(g + 1) * HEADS_PC]).reshape(
                    HEADS_PC * H, D).astype(BF16),
            "cc": CC[b].astype(BF16),
            "ss": SS[b].astype(BF16),
        }
        if apply_w:
            m["qw"] = np.ascontiguousarray(np.broadcast_to(
                (1.0 + q_norm_w).astype(np.float32), (128, H)))
            m["kw"] = np.ascontiguousarray(np.broadcast_to(
                (1.0 + k_norm_w).astype(np.float32), (128, H)))
        if mode == "arbitrary":
            m["am"] = np.where(attn_mask[b], np.float32(0.0),
                               np.float32(K_MASK)).astype(np.float32)
        in_maps.append(m)

    res = bass_utils.run_bass_kernel_spmd(nc, in_maps, core_ids=list(range(8)))
    LAST_RESULT = res
    LAST_IN_MAPS = in_maps
    out = np.zeros((B, T, D), np.float32)
    for c in range(8):
        out[c // 4] += res.results[c]["out"].astype(np.float32)
    return out



# revision 9
# speedup vs baseline: 6.8550x; 6.8550x over previous
"""Trainium2 Bass kernel for GQA attention with QK-RMSNorm, partial mRoPE,
causal mask and sigmoid output gate (nn_Attention_70557722739259).

Model dims: B=2, T=2048, D=2048, N=16 Q heads, K=2 KV heads, H=256.
Sharding over 8 NeuronCores: DP=2 over batch x TP=4 over head groups
(4 Q heads + their shared KV head per core). Each core computes a partial
output projection over its 4 heads; the host sums the 4 partials per batch
(Megatron-style unshard).

All device inputs are host-packed into [128, X] layouts so each tensor
loads in 1-4 large DMAs (the DMA queue is the scarce resource, not bytes).

Per-core device algorithm (all matmuls bf16, softmax f32):
  phase 1 (single full-T pass per projection): k/v then per q-head
           projections from host-pre-packed x^T; k/v/q extracted straight
           from PSUM (Pool-engine copies), RMSNorm stats via one DVE
           tensor_tensor_reduce per tile, sigmoid gate on Act from PSUM,
           per-head batched Sqrt for rms, RoPE + norm scale token-major,
           PE-transpose Q,K to feature-major (H, T). Each head's RoPE
           (DVE) overlaps the next head's projection matmuls (PE).
  phase 2: per head, S^T = K^T.T @ Q^T tiles (keys on partitions), exp via
           ACT (1/16 scale folded in; no max subtraction - scores are O(10)),
           diagonal-block masking via Pool affine_select on the first 128
           columns only (fully-masked sub-blocks are never computed: the
           moving operand is narrowed on the diagonal strip), AV + softmax
           denominator accumulated in PSUM via ones-column in V. The i-loop
           is software-pipelined depth 3 (scores for i+3 issue before AV of
           i) so exp latency never stalls PE. Head flushes are split: the
           DVE renorm/gate chain issues right after the AV loop (it runs
           under the interleaved o-proj), the PE transposes are deferred
           past the next head's score prologue.
  phase 3: o-proj partial from gated qkv^T, interleaved into phase 2 with a
           one-block lag to keep PE busy during per-head flushes.
  PSUM: tag "big" (4 bufs: projections, scores, o-proj) + av0..av3
  accumulators (also reused as transpose targets) = exactly 8 banks.
"""
import sys
sys.path.insert(0, "/opt/trn_rl_repo")
import numpy as np
import ml_dtypes

from concourse import bacc, tile, mybir
from concourse import bass_utils
from concourse.masks import make_identity

BF16 = ml_dtypes.bfloat16
F32 = mybir.dt.float32
BF = mybir.dt.bfloat16

B, T, D = 2, 2048, 2048
N_HEADS, N_KV, H = 16, 2, 256
HEADS_PC = 4            # q heads per core (TP=4)
ROPE_THETA = 1000000
ROTARY = 64             # int(H * 0.25)
FREQ = 32
NORM_EPS = 1e-6
K_MASK = -2.3819763e38
SCALE = H ** (-0.5)     # 1/16

TT = T // 128           # 16 token tiles
DC = D // 128           # 16 contraction chunks
PIPE = 3                # phase-2 score/exp software-pipeline depth

LAST_RESULT = None
LAST_IN_MAPS = None
_COMPILED = {}


def _build(mode="causal", apply_w=False):
    nc = bacc.Bacc("TRN2", target_bir_lowering=False, debug=False,
                   enable_asserts=True, num_devices=8)
    Act = mybir.ActivationFunctionType
    Alu = mybir.AluOpType

    # host-packed layouts (see kernel() for the packing):
    #   xT  [128, 32768]: col = cs*8192 + d*512 + tl4*128 + t   (t tile ti = cs*4+tl4)
    #   wq  [128, 32768]: col = h*8192 + d*512 + f
    #   wkv [128, 8192]:  col = d*512 + f                        (f: k 0:256, v 256:512)
    #   wo  [128, 16384]: col = c*2048 + dcol                    (c: 8 feature chunks)
    #   cc/ss [128, 1024]: col = ti*64 + r
    xT = nc.dram_tensor("xT", (128, 16 * 2048), BF, kind="ExternalInput").ap()
    wq = nc.dram_tensor("wq", (128, 16 * 2048), BF, kind="ExternalInput").ap()
    wkv = nc.dram_tensor("wkv", (128, 16 * 512), BF, kind="ExternalInput").ap()
    wo = nc.dram_tensor("wo", (128, 8 * 2048), BF, kind="ExternalInput").ap()
    cc = nc.dram_tensor("cc", (128, TT * ROTARY), BF, kind="ExternalInput").ap()
    ss = nc.dram_tensor("ss", (128, TT * ROTARY), BF, kind="ExternalInput").ap()
    if apply_w:
        qw = nc.dram_tensor("qw", (128, H), F32, kind="ExternalInput").ap()
        kw = nc.dram_tensor("kw", (128, H), F32, kind="ExternalInput").ap()
    if mode == "arbitrary":
        am = nc.dram_tensor("am", (T, T), F32, kind="ExternalInput").ap()
    out = nc.dram_tensor("out", (T, D), BF, kind="ExternalOutput").ap()

    causal = (mode == "causal")

    def xcol(d, ti):
        cs, tl4 = ti // 4, ti % 4
        return cs * 8192 + d * 512 + tl4 * 128

    with tile.TileContext(nc) as tc:
        with tc.tile_pool(name="const", bufs=1) as constp, \
             tc.tile_pool(name="attn", bufs=1) as ap_, \
             tc.tile_pool(name="psum", bufs=1, space="PSUM") as psum:

            # ---- constants ----
            ident = constp.tile([128, 128], BF, tag="ident", name="ident")
            make_identity(nc, ident[:])
            epst = constp.tile([128, 1], F32, tag="epst", name="epst")
            nc.gpsimd.memset(epst[:], NORM_EPS)
            if apply_w:
                qw_sb = constp.tile([128, H], F32, tag="qw", name="qw")
                kw_sb = constp.tile([128, H], F32, tag="kw", name="kw")
                nc.sync.dma_start(qw_sb[:], qw[:])
                nc.sync.dma_start(kw_sb[:], kw[:])

            # ---- persistent attention tensors (span phase 1 -> 2) ----
            QT = [[ap_.tile([128, T], BF, tag=f"QT{h}_{c}", name=f"QT{h}_{c}")
                   for c in range(2)] for h in range(HEADS_PC)]
            KT = [ap_.tile([128, T], BF, tag=f"KT{c}", name=f"KT{c}")
                  for c in range(2)]
            V = [ap_.tile([128, H + 1], BF, tag=f"V{i}", name=f"V{i}")
                 for i in range(TT)]
            for i in range(TT):
                nc.gpsimd.memset(V[i][:, H:H + 1], 1.0)
            gate = [[ap_.tile([128, H], BF, tag=f"g{h}_{i}", name=f"g{h}_{i}")
                     for i in range(TT)] for h in range(HEADS_PC)]

            tp_ctr = [0]            # rotates transpose targets over av0..av3

            def transpose_128(src_ap, dst_ap, nm):
                """PE-transpose a [128,128] bf16 block via an av psum slot."""
                tag = f"av{tp_ctr[0] % 4}"
                tp_ctr[0] += 1
                tp = psum.tile([128, 128], BF, tag=tag, name=f"tp{nm}")
                nc.tensor.transpose(tp[:], src_ap, ident[:])
                nc.any.tensor_copy(dst_ap, tp[:])

            # ================= phase 1 =================
            with tc.tile_pool(name="praw", bufs=1) as praw, \
                 tc.tile_pool(name="proj", bufs=1) as proj, \
                 tc.tile_pool(name="p1c", bufs=3) as p1c:
                # DMA order = need order: wkv, xT[cs0], wq[h0], xT[cs1..3],
                # wq[h1], cc, ss; wq[h2], wq[h3] issued in the head loop.
                xT_sb = proj.tile([128, 16 * 2048], BF, tag="xT", name="xT_sb")
                wkv_sb = proj.tile([128, 16 * 512], BF, tag="wqh", bufs=2,
                                   name="wkv_sb")
                for q_ in range(2):
                    nc.sync.dma_start(wkv_sb[:, q_ * 4096:(q_ + 1) * 4096],
                                      wkv[:, q_ * 4096:(q_ + 1) * 4096])
                for q_ in range(2):
                    nc.sync.dma_start(xT_sb[:, q_ * 4096:(q_ + 1) * 4096],
                                      xT[:, q_ * 4096:(q_ + 1) * 4096])
                wq_sb = [None] * HEADS_PC

                def load_wq(h):
                    w_ = proj.tile([128, 16 * 512], BF, tag="wqh", bufs=2,
                                   name=f"wq_sb{h}")
                    for q_ in range(2):
                        nc.sync.dma_start(
                            w_[:, q_ * 4096:(q_ + 1) * 4096],
                            wq[:, h * 8192 + q_ * 4096:
                               h * 8192 + (q_ + 1) * 4096])
                    wq_sb[h] = w_

                load_wq(0)
                for q_ in range(2, 8):
                    nc.sync.dma_start(xT_sb[:, q_ * 4096:(q_ + 1) * 4096],
                                      xT[:, q_ * 4096:(q_ + 1) * 4096])
                load_wq(1)
                cc_sb = praw.tile([128, TT * ROTARY], BF, tag="cc", name="cc_sb")
                ss_sb = praw.tile([128, TT * ROTARY], BF, tag="ss", name="ss_sb")
                nc.sync.dma_start(cc_sb[:], cc[:])
                nc.sync.dma_start(ss_sb[:], ss[:])

                # ---- rope + norm scale + transpose for one token tile ----
                def rope_pass(raw, rinv_ap, dst_tiles, ti, w_sb, nm):
                    src = raw
                    if apply_w:
                        srw = p1c.tile([128, H], F32, tag="srw", name=f"srw{nm}")
                        nc.vector.tensor_mul(srw[:], raw[:], w_sb[:])
                        src = srw
                    ccs = cc_sb[:, ti * ROTARY:(ti + 1) * ROTARY]
                    sss = ss_sb[:, ti * ROTARY:(ti + 1) * ROTARY]
                    rot = p1c.tile([128, ROTARY], BF, tag="rot", name=f"rot{nm}")
                    t2 = p1c.tile([128, ROTARY], BF, tag="rot2", name=f"rot2{nm}")
                    nc.vector.tensor_mul(rot[:], src[:, 0:ROTARY], ccs)
                    nc.vector.tensor_mul(t2[:, 0:FREQ], src[:, FREQ:ROTARY],
                                         sss[:, 0:FREQ])
                    nc.vector.tensor_mul(t2[:, FREQ:ROTARY], src[:, 0:FREQ],
                                         sss[:, FREQ:ROTARY])
                    nc.vector.tensor_add(rot[:], rot[:], t2[:])
                    tok = p1c.tile([128, H], BF, tag="tok", name=f"tok{nm}")
                    nc.vector.tensor_scalar_mul(tok[:, 0:ROTARY], rot[:],
                                                rinv_ap)
                    nc.vector.tensor_scalar_mul(tok[:, ROTARY:H],
                                                src[:, ROTARY:H], rinv_ap)
                    for c2 in range(2):
                        transpose_128(tok[:, c2 * 128:(c2 + 1) * 128],
                                      dst_tiles[c2][:, ti * 128:(ti + 1) * 128],
                                      f"{nm}_{c2}")

                # ---- k/v pass over all token tiles ----
                k_raw = [praw.tile([128, H], BF, tag=f"kr{ti}", name=f"kr{ti}")
                         for ti in range(TT)]
                ssqk = praw.tile([128, TT], F32, tag="ssqk", name="ssqk")
                for ti in range(TT):
                    pk = psum.tile([128, 2 * H], F32, tag="big", bufs=4,
                                   name=f"pk{ti}")
                    for d in range(DC):
                        nc.tensor.matmul(
                            pk[:], xT_sb[:, xcol(d, ti):xcol(d, ti) + 128],
                            wkv_sb[:, d * 512:(d + 1) * 512],
                            start=(d == 0), stop=(d == DC - 1))
                    nc.vector.tensor_copy(k_raw[ti][:], pk[:, 0:H])
                    nc.scalar.copy(V[ti][:, 0:H], pk[:, H:2 * H])
                    junk = p1c.tile([128, H], F32, tag="junk", name=f"jk{ti}")
                    nc.scalar.square(junk[:], k_raw[ti][:])
                    nc.vector.reduce_sum(ssqk[:, ti:ti + 1], junk[:],
                                         axis=mybir.AxisListType.X)
                rmsk = praw.tile([128, TT], F32, tag="rmsk", name="rmsk")
                rinvk = praw.tile([128, TT], F32, tag="rinvk", name="rinvk")
                nc.scalar.activation(rmsk[:], ssqk[:], Act.Sqrt,
                                     scale=1.0 / H, bias=epst[:])
                nc.vector.reciprocal(rinvk[:], rmsk[:])
                for ti in range(TT):
                    rope_pass(k_raw[ti], rinvk[:, ti:ti + 1], KT, ti,
                              kw_sb if apply_w else None, f"k{ti}")

                # ---- per-head q pass; RoPE of head h overlaps head h+1 ----
                for h in range(HEADS_PC):
                    if h >= 2:
                        load_wq(h)
                    q_raw = [praw.tile([128, H], BF, tag=f"qr{ti}", bufs=1,
                                       name=f"qr{h}_{ti}") for ti in range(TT)]
                    ssqq = praw.tile([128, TT], F32, tag="ssqq", bufs=2,
                                     name=f"ssqq{h}")
                    for ti in range(TT):
                        pq = psum.tile([128, 2 * H], F32, tag="big", bufs=4,
                                       name=f"pq{h}_{ti}")
                        for d in range(DC):
                            nc.tensor.matmul(
                                pq[:], xT_sb[:, xcol(d, ti):xcol(d, ti) + 128],
                                wq_sb[h][:, d * 512:(d + 1) * 512],
                                start=(d == 0), stop=(d == DC - 1))
                        nc.vector.tensor_copy(q_raw[ti][:], pq[:, 0:H])
                        nc.scalar.activation(gate[h][ti][:], pq[:, H:2 * H],
                                             Act.Sigmoid)
                        junk = p1c.tile([128, H], F32, tag="junk",
                                        name=f"jq{h}_{ti}")
                        nc.scalar.square(junk[:], q_raw[ti][:])
                        nc.vector.reduce_sum(ssqq[:, ti:ti + 1], junk[:],
                                             axis=mybir.AxisListType.X)
                    rmsq = praw.tile([128, TT], F32, tag="rmsq", bufs=2,
                                     name=f"rmsq{h}")
                    rinvq = praw.tile([128, TT], F32, tag="rinvq", bufs=2,
                                      name=f"rinvq{h}")
                    nc.scalar.activation(rmsq[:], ssqq[:], Act.Sqrt,
                                         scale=1.0 / H, bias=epst[:])
                    nc.vector.reciprocal(rinvq[:], rmsq[:])
                    for ti in range(TT):
                        rope_pass(q_raw[ti], rinvq[:, ti:ti + 1], QT[h], ti,
                                  qw_sb if apply_w else None, f"q{h}_{ti}")

            # ================= phases 2+3 =================
            with tc.tile_pool(name="p23", bufs=1) as p23, \
                 tc.tile_pool(name="p2w", bufs=3) as p2w:
                qkvgT = [p23.tile([128, T], BF, tag=f"qkT{c}", name=f"qkT{c}")
                         for c in range(2 * HEADS_PC)]
                wo_sb = p23.tile([128, 8 * 2048], BF, tag="wo", name="wo_sb")
                for q_ in range(4):
                    nc.sync.dma_start(wo_sb[:, q_ * 4096:(q_ + 1) * 4096],
                                      wo[:, q_ * 4096:(q_ + 1) * 4096])

                def emit_po(ti):
                    """Phase-3 o-proj partial for one token tile."""
                    ot = p2w.tile([128, 2048], BF, tag="ot", bufs=2,
                                  name=f"ot{ti}")
                    for db in range(4):
                        po = psum.tile([128, 512], F32, tag="big", bufs=4,
                                       name=f"po{ti}_{db}")
                        for c in range(2 * HEADS_PC):
                            nc.tensor.matmul(
                                po[:], qkvgT[c][:, ti * 128:(ti + 1) * 128],
                                wo_sb[:, c * 2048 + db * 512:
                                      c * 2048 + (db + 1) * 512],
                                start=(c == 0), stop=(c == 2 * HEADS_PC - 1))
                        if db % 2 == 0:
                            nc.scalar.copy(ot[:, db * 512:(db + 1) * 512],
                                           po[:])
                        else:
                            nc.vector.tensor_copy(
                                ot[:, db * 512:(db + 1) * 512], po[:])
                    nc.sync.dma_start(out[ti * 128:(ti + 1) * 128, :], ot[:])

                tps_prev = None         # deferred PE transposes of last flush
                for j in range(4):              # q blocks of 512
                    kmax = 4 * (j + 1) if causal else TT
                    for h in range(HEADS_PC):
                        # scores + exp for key block i (narrowed on diagonal)
                        def issue_st(i, nm, h=h, j=j):
                            dlt = max(0, i - 4 * j) if causal else 0
                            w = 512 - 128 * dlt
                            st = psum.tile([128, 512], F32, tag="big", bufs=4,
                                           name=f"st{nm}")
                            q0 = j * 512 + dlt * 128
                            for c2 in range(2):
                                nc.tensor.matmul(
                                    st[:, 0:w],
                                    KT[c2][:, i * 128:(i + 1) * 128],
                                    QT[h][c2][:, q0:(j + 1) * 512],
                                    start=(c2 == 0), stop=(c2 == 1))
                            if mode == "arbitrary":
                                amt = p2w.tile([128, 512], F32, tag="amt",
                                               name=f"am{nm}")
                                nc.sync.dma_start(
                                    amt[:], am[i * 128:(i + 1) * 128,
                                               j * 512:(j + 1) * 512])
                                nc.vector.tensor_add(st[:], st[:], amt[:])
                            pT = p2w.tile([128, 512], BF, tag="pT",
                                          name=f"pT{nm}")
                            nc.scalar.activation(pT[:, 0:w], st[:, 0:w],
                                                 Act.Exp, scale=SCALE)
                            if causal and i >= 4 * j:
                                nc.gpsimd.affine_select(
                                    out=pT[:, 0:128], in_=pT[:, 0:128],
                                    compare_op=Alu.is_ge, fill=0.0,
                                    base=0, channel_multiplier=-1,
                                    pattern=[[1, 128]])
                            return pT, dlt

                        # prologue scores, then deferred transposes of the
                        # previous head's flush (their DVE inputs are ready)
                        pts = {}
                        for i in range(min(PIPE, kmax)):
                            pts[i] = issue_st(i, f"{h}_{j}_{i}")
                        if tps_prev is not None:
                            tps_prev()
                            tps_prev = None
                        av = [psum.tile([128, H + 1], F32, tag=f"av{s}",
                                        name=f"av{h}_{j}_{s}")
                              for s in range(4)]
                        for i in range(kmax):
                            if i + PIPE < kmax:
                                pts[i + PIPE] = issue_st(i + PIPE,
                                                         f"{h}_{j}_{i+PIPE}")
                            pT, dlt = pts.pop(i)
                            for s in range(dlt, 4):
                                last_i = (4 * j + s) if causal else (kmax - 1)
                                if i > last_i:
                                    continue
                                nc.tensor.matmul(
                                    av[s][:],
                                    pT[:, (s - dlt) * 128:(s - dlt + 1) * 128],
                                    V[i][:], start=(i == 0),
                                    stop=(i == last_i))

                        # phase 3 with one-block lag keeps PE busy here
                        if j > 0:
                            emit_po(4 * (j - 1) + h)

                        # flush DVE chain now (runs under the o-proj);
                        # PE transposes deferred past next head's prologue
                        qks = []
                        for s in range(4):
                            ti = 4 * j + s
                            rec = p2w.tile([128, 1], F32, tag="rec",
                                           name=f"rec{h}_{ti}")
                            if mode == "arbitrary":
                                dcl = p2w.tile([128, 1], F32, tag="dcl",
                                               name=f"dcl{h}_{ti}")
                                nc.vector.tensor_scalar_max(
                                    dcl[:], av[s][:, H:H + 1], 1e-30)
                                nc.vector.reciprocal(rec[:], dcl[:])
                            else:
                                nc.vector.reciprocal(rec[:], av[s][:, H:H + 1])
                            qk0 = p2w.tile([128, H], BF, tag="qk0",
                                           name=f"qk0{h}_{ti}")
                            nc.vector.tensor_mul(qk0[:], av[s][:, 0:H],
                                                 gate[h][ti][:])
                            qk = p2w.tile([128, H], BF, tag="qkg",
                                          name=f"qkg{h}_{ti}")
                            nc.vector.tensor_scalar_mul(qk[:], qk0[:], rec[:])
                            qks.append(qk)

                        def tps(qks=qks, h=h, j=j):
                            for s in range(4):
                                ti = 4 * j + s
                                for c2 in range(2):
                                    transpose_128(
                                        qks[s][:, c2 * 128:(c2 + 1) * 128],
                                        qkvgT[2 * h + c2][
                                            :, ti * 128:(ti + 1) * 128],
                                        f"o{h}_{ti}_{c2}")
                        tps_prev = tps

                tps_prev()
                # trailing o-proj for the last query block
                for ti in range(4 * 3, TT):
                    emit_po(ti)

    nc.compile()
    return nc


def _get_compiled(mode, apply_w):
    key = (mode, apply_w)
    if key not in _COMPILED:
        _COMPILED[key] = _build(mode, apply_w)
    return _COMPILED[key]


def _rope_tables(positions):
    """Host: exact reference mRoPE sin/cos tables -> CC=[cos|cos], SS=[-sin|sin]."""
    fraction = 2.0 * np.arange(FREQ, dtype=np.float32) / ROTARY
    timescale = (ROPE_THETA ** fraction).astype(np.float32)
    CC, SS = [], []
    for b in range(positions.shape[1]):
        sinusoid = positions[:, b, :, None].astype(np.float32) / timescale
        freq = sinusoid[0].copy()
        h_idx = np.arange(1, 11 * 3, 3)
        w_idx = np.arange(2, 10 * 3, 3)
        freq[:, h_idx] = sinusoid[1][:, h_idx]
        freq[:, w_idx] = sinusoid[2][:, w_idx]
        sin, cos = np.sin(freq), np.cos(freq)
        CC.append(np.concatenate([cos, cos], axis=1).astype(np.float32))
        SS.append(np.concatenate([-sin, sin], axis=1).astype(np.float32))
    return CC, SS


def _pack_rows(arr, blk=128):
    """[R, C] with R = n*128 -> [128, n*C] (row-chunk-major columns)."""
    r, c = arr.shape
    n = r // blk
    return np.ascontiguousarray(
        arr.reshape(n, blk, c).transpose(1, 0, 2).reshape(blk, n * c))


def _pack_xT(xt):
    """[D, T] -> [128, 32768] with col = cs*8192 + d*512 + tl4*128 + t."""
    # (16d, 128p, 4cs, 4tl4, 128t) -> (128p, 4cs, 16d, 4tl4, 128t)
    v = xt.reshape(DC, 128, 4, 4, 128).transpose(1, 2, 0, 3, 4)
    return np.ascontiguousarray(v.reshape(128, DC * 2048))


def kernel(x, positions, attn_mask, wq, wk, wv, wo, q_norm_w, k_norm_w):
    global LAST_RESULT, LAST_IN_MAPS
    x = np.asarray(x)
    positions = np.asarray(positions)
    attn_mask = np.asarray(attn_mask)
    wq, wk, wv, wo = map(np.asarray, (wq, wk, wv, wo))
    q_norm_w, k_norm_w = np.asarray(q_norm_w), np.asarray(k_norm_w)

    tril = np.tril(np.ones((T, T), dtype=bool))
    if all(np.array_equal(attn_mask[b], tril) for b in range(B)):
        mode = "causal"
    elif attn_mask.all():
        mode = "full"
    else:
        mode = "arbitrary"
    apply_w = bool(np.any(q_norm_w != 0) or np.any(k_norm_w != 0))

    nc = _get_compiled(mode, apply_w)
    CC, SS = _rope_tables(positions)
    group = N_HEADS // N_KV  # q heads per kv head = 8

    in_maps = []
    for c in range(8):
        b, g = c // 4, c % 4
        kvh = (g * HEADS_PC) // group
        wq_g = wq[:, g * HEADS_PC:(g + 1) * HEADS_PC, :]    # (D, 4, 512)
        wq_packed = np.concatenate(
            [_pack_rows(np.ascontiguousarray(wq_g[:, h, :]))
             for h in range(HEADS_PC)], axis=1)             # (128, 32768)
        m = {
            "xT": _pack_xT(np.ascontiguousarray(x[b].T)).astype(BF16),
            "wq": wq_packed.astype(BF16),
            "wkv": _pack_rows(np.concatenate(
                [wk[:, kvh, :], wv[:, kvh, :]], axis=1)).astype(BF16),
            "wo": _pack_rows(
                wo[g * HEADS_PC:(g + 1) * HEADS_PC].reshape(
                    HEADS_PC * H, D)).astype(BF16),
            "cc": _pack_rows(CC[b]).astype(BF16),
            "ss": _pack_rows(SS[b]).astype(BF16),
        }
        if apply_w:
            m["qw"] = np.ascontiguousarray(np.broadcast_to(
                (1.0 + q_norm_w).astype(np.float32), (128, H)))
            m["kw"] = np.ascontiguousarray(np.broadcast_to(
                (1.0 + k_norm_w).astype(np.float32), (128, H)))
        if mode == "arbitrary":
            m["am"] = np.where(attn_mask[b], np.float32(0.0),
                               np.float32(K_MASK)).astype(np.float32)
        in_maps.append(m)

    res = bass_utils.run_bass_kernel_spmd(nc, in_maps, core_ids=list(range(8)))
    LAST_RESULT = res
    LAST_IN_MAPS = in_maps
    out = np.zeros((B, T, D), np.float32)
    for c in range(8):
        out[c // 4] += res.results[c]["out"].astype(np.float32)
    return out
